# revision 11
# baseline (speedup 1.0000x reference)
"""PointerGenerator Bass kernel for 8 TRN2 cores.

Math (per row r = (batch b, query q)):
  attn       = softmax(x_r . C_b^T / 16)                       [512]
  logits     = x_r @ W + b_state                               [32000]
  lse        = logsumexp(logits)
  g          = sigmoid([x; ctx_vec; dom] @ W_gen + b_gen)
  out dense  = logit_v + (log g - lse)            (p_ptr = 0 columns)
  out touched= log(g*exp(logit_v - lse) + (1-g)*pptr_v)  at v = ids[b,k]

Sharding: pure data parallel, 2 batches (128 rows) per core. Dense part is a
[128,32000] affine map of the logits (written bf16); the sparse correction is
computed on-device for the <=512 touched columns per row and emitted as a second
[128,512] f32 output. The host places the corrections during unshard (HW's
indirect-DMA scatter is one-offset-per-partition, so elementwise scatter is not
expressible on-device; duplicate ids carry bit-identical values so placement
order is irrelevant).
"""

import os

import numpy as np
import ml_dtypes

from concourse import bass, mybir, bass_utils
import concourse.tile as tile
from concourse.masks import make_identity

BF16 = ml_dtypes.bfloat16
B, TQ, TK, D, V = 16, 64, 512, 256, 32000
P = 128
TS = 500          # dense column tile size
NT = V // TS      # 64 dense tiles
NG = 8            # W load groups (8 tiles each)

LAST_RESULTS = None
_PROG_CACHE = {}


def build_program(bias_nonzero=False, debug=False):
    from concourse import bacc

    nc = bacc.Bacc(None, target_bir_lowering=False, debug=debug)
    dt = mybir.dt
    f32, bf16, i32 = dt.float32, dt.bfloat16, dt.int32
    AF = mybir.ActivationFunctionType
    ALU = mybir.AluOpType

    xa_d = nc.dram_tensor("xa", [P, 3, P], bf16, kind="ExternalInput")
    dom_d = nc.dram_tensor("domT", [P, 2, P], bf16, kind="ExternalInput")
    ctxT_d = nc.dram_tensor("ctxT", [P, 2, 2, TK], bf16, kind="ExternalInput")
    ctx_d = nc.dram_tensor("ctx", [P, 2, 4, D], bf16, kind="ExternalInput")
    w2_d = nc.dram_tensor("w2", [P, 2, V], bf16, kind="ExternalInput")
    wta_d = nc.dram_tensor("wta", [V, 384], bf16, kind="ExternalInput")
    idsf_d = nc.dram_tensor("idsf", [P, 8], f32, kind="ExternalInput")
    idsi_d = nc.dram_tensor("idsi", [P, 8], i32, kind="ExternalInput")
    wg_d = nc.dram_tensor("wg", [P, 6], bf16, kind="ExternalInput")
    bgh_d = nc.dram_tensor("bgh", [P, 1], f32, kind="ExternalInput")
    if bias_nonzero:
        brow_d = nc.dram_tensor("brow", [NT, TS], bf16, kind="ExternalInput")
    out_d = nc.dram_tensor("out", [P, V], bf16, kind="ExternalOutput")
    corr_d = nc.dram_tensor("corr_out", [P, TK], f32, kind="ExternalOutput")

    with tile.TileContext(nc) as tc:
        with (
            tc.tile_pool(name="cpool", bufs=1) as cpool,
            tc.tile_pool(name="work", bufs=1) as work,
            tc.tile_pool(name="mcp", bufs=2) as mcp,
            tc.tile_pool(name="gwp", bufs=2) as gwp,
            tc.tile_pool(name="wtp", bufs=2) as wtp,
            tc.tile_pool(name="esp", bufs=2) as esp,
            tc.tile_pool(name="otp", bufs=2) as otp,
            tc.tile_pool(name="pdp", bufs=2, space="PSUM") as pdp,
            tc.tile_pool(name="ptp", bufs=2, space="PSUM") as ptp,
            tc.tile_pool(name="pmp", bufs=1, space="PSUM") as pmp,
        ):
            # ---------------- constant loads ----------------
            xa = cpool.tile([P, 3, P], bf16, name="xa_t")
            nc.sync.dma_start(out=xa[:], in_=xa_d[:])
            domT = cpool.tile([P, 2, P], bf16, name="domT_t")
            nc.sync.dma_start(out=domT[:], in_=dom_d[:])
            ctxT = cpool.tile([P, 2, 2, TK], bf16, name="ctxT_t")
            nc.sync.dma_start(out=ctxT[:], in_=ctxT_d[:])
            ctxs = cpool.tile([P, 2, 4, D], bf16, name="ctxs_t")
            nc.sync.dma_start(out=ctxs[:], in_=ctx_d[:])
            idsf = cpool.tile([P, 8], f32, name="idsf_t")
            nc.sync.dma_start(out=idsf[:], in_=idsf_d[:])
            idsi = cpool.tile([P, 8], i32, name="idsi_t")
            nc.sync.dma_start(out=idsi[:], in_=idsi_d[:])
            wg = cpool.tile([P, 6], bf16, name="wg_t")
            nc.sync.dma_start(out=wg[:], in_=wg_d[:])
            bgh = cpool.tile([P, 1], f32, name="bgh_t")
            nc.sync.dma_start(out=bgh[:], in_=bgh_d[:])
            if bias_nonzero:
                brow = cpool.tile([NT, TS], bf16, name="brow_t")
                nc.sync.dma_start(out=brow[:], in_=brow_d[:])
                onesM = cpool.tile([P, P], bf16, name="onesM")
                nc.gpsimd.memset(onesM[:], 1.0)

            ident_f = cpool.tile([P, P], f32, name="ident_f")
            make_identity(nc, ident_f[:])
            ident_b = cpool.tile([P, P], bf16, name="ident_b")
            make_identity(nc, ident_b[:])

            # ---------------- attention scores + softmax ----------------
            ps = pmp.tile([P, TK], f32, name="ps", tag="ps")
            for b in range(2):
                bc = slice(b * 64, (b + 1) * 64)
                for kc in range(2):
                    nc.tensor.matmul(
                        out=ps[bc, :],
                        lhsT=xa[:, kc, bc],
                        rhs=ctxT[:, kc, b, :],
                        start=(kc == 0),
                        stop=(kc == 1),
                    )
            attn_exp = work.tile([P, TK], f32, name="attn_exp")
            arow = work.tile([P, 1], f32, name="arow")
            nc.scalar.activation(
                out=attn_exp[:], in_=ps[:], func=AF.Exp,
                scale=1.0 / 16.0, accum_out=arow[:],
            )
            arec = work.tile([P, 1], f32, name="arec")
            nc.vector.reciprocal(arec[:], arow[:])
            attn_bf = work.tile([P, TK], bf16, name="attn_bf")
            nc.vector.tensor_scalar(
                out=attn_bf[:], in0=attn_exp[:],
                scalar1=arec[:, :1], scalar2=None, op0=ALU.mult,
            )

            # ---------------- attn transposed [key, query] ----------------
            attnT = work.tile([P, 2, 4, 64], bf16, name="attnT")
            for b in range(2):
                bc = slice(b * 64, (b + 1) * 64)
                ib = slice(b * 64, b * 64 + 64)
                for c in range(4):
                    tp = ptp.tile([P, 64], bf16, name="tp", tag="tp")
                    nc.tensor.transpose(
                        out=tp[:],
                        in_=attn_bf[bc, c * 128:(c + 1) * 128],
                        identity=ident_b[ib, ib],
                    )
                    nc.vector.tensor_copy(out=attnT[:, b, c, :], in_=tp[:])

            # ---------------- row-replicated ids ----------------
            idsrow = work.tile([P, 2, TK], f32, name="idsrow")
            for b in range(2):
                for c in range(4):
                    col = b * 4 + c
                    tp = ptp.tile([P, P], f32, name="tp", tag="tp")
                    nc.tensor.transpose(
                        out=tp[:],
                        in_=idsf[:, col:col + 1].to_broadcast([P, P]),
                        identity=ident_f[:],
                    )
                    nc.vector.tensor_copy(
                        out=idsrow[:, b, c * 128:(c + 1) * 128], in_=tp[:]
                    )

            # ------- pptr at touched cols: attn @ (ids==ids') ----------
            pp = pmp.tile([P, TK], f32, name="pp", tag="pp")
            for b in range(2):
                bc = slice(b * 64, (b + 1) * 64)
                for c in range(4):
                    col = b * 4 + c
                    mc = mcp.tile([P, TK], bf16, name="mc")
                    nc.vector.tensor_scalar(
                        out=mc[:], in0=idsrow[:, b, :],
                        scalar1=idsf[:, col:col + 1], scalar2=None,
                        op0=ALU.is_equal,
                    )
                    nc.tensor.matmul(
                        out=pp[bc, :], lhsT=attnT[:, b, c, :], rhs=mc[:],
                        start=(c == 0), stop=(c == 3),
                    )

            # ---------------- context vector + gate ----------------
            cv = pmp.tile([P, D], f32, name="cv", tag="seq")
            for b in range(2):
                bc = slice(b * 64, (b + 1) * 64)
                for c in range(4):
                    nc.tensor.matmul(
                        out=cv[bc, :], lhsT=attnT[:, b, c, :],
                        rhs=ctxs[:, b, c, :],
                        start=(c == 0), stop=(c == 3),
                    )
            cvec_sb = work.tile([P, D], bf16, name="cvec_sb")
            nc.vector.tensor_copy(out=cvec_sb[:], in_=cv[:])

            cvecT = work.tile([P, 2, 2, 64], bf16, name="cvecT")
            for b in range(2):
                bc = slice(b * 64, (b + 1) * 64)
                ib = slice(b * 64, b * 64 + 64)
                for kc in range(2):
                    tp = ptp.tile([P, 64], bf16, name="tp", tag="tp")
                    nc.tensor.transpose(
                        out=tp[:],
                        in_=cvec_sb[bc, kc * 128:(kc + 1) * 128],
                        identity=ident_b[ib, ib],
                    )
                    nc.vector.tensor_copy(out=cvecT[:, b, kc, :], in_=tp[:])

            gp = pmp.tile([P, 1], f32, name="gp", tag="seq")
            for b in range(2):
                bc = slice(b * 64, (b + 1) * 64)
                lhs = [
                    xa[:, 0, bc], xa[:, 1, bc],
                    cvecT[:, b, 0, :], cvecT[:, b, 1, :],
                    domT[:, 0, bc], domT[:, 1, bc],
                ]
                for j, lh in enumerate(lhs):
                    nc.tensor.matmul(
                        out=gp[bc, :], lhsT=lh, rhs=wg[:, j:j + 1],
                        start=(j == 0), stop=(j == 5),
                    )
            g = work.tile([P, 1], f32, name="g")
            nc.scalar.activation(out=g[:], in_=gp[:], func=AF.Sigmoid, bias=bgh[:, :1])
            logg = work.tile([P, 1], f32, name="logg")
            nc.scalar.activation(out=logg[:], in_=g[:], func=AF.Ln)
            gm1 = work.tile([P, 1], f32, name="gm1")
            nc.vector.tensor_scalar(
                out=gm1[:], in0=g[:], scalar1=-1.0, scalar2=1.0,
                op0=ALU.mult, op1=ALU.add,
            )

            # -------- gather W columns at touched ids; logits there --------
            wcolsT = work.tile([P, 2, 3, TK], bf16, name="wcolsT")
            for b in range(2):
                for c in range(4):
                    col = b * 4 + c
                    gw = gwp.tile([P, 384], bf16, name="gw")
                    nc.gpsimd.indirect_dma_start(
                        out=gw[:],
                        out_offset=None,
                        in_=wta_d[:],
                        in_offset=bass.IndirectOffsetOnAxis(
                            ap=idsi[:, col:col + 1], axis=0
                        ),
                    )
                    for f in range(3):
                        tp = ptp.tile([P, P], bf16, name="tp", tag="tp")
                        nc.tensor.transpose(
                            out=tp[:],
                            in_=gw[:, f * 128:(f + 1) * 128],
                            identity=ident_b[:],
                        )
                        nc.vector.tensor_copy(
                            out=wcolsT[:, b, f, c * 128:(c + 1) * 128], in_=tp[:]
                        )
            lt = pmp.tile([P, TK], f32, name="lt", tag="seq")
            for b in range(2):
                bc = slice(b * 64, (b + 1) * 64)
                for f in range(3):
                    nc.tensor.matmul(
                        out=lt[bc, :], lhsT=xa[:, f, bc], rhs=wcolsT[:, b, f, :],
                        start=(f == 0), stop=(f == 2),
                    )

            # ---------------- dense logits + exp-sums ----------------
            logits_sb = work.tile([P, V], bf16, name="logits_sb")
            rsparts = work.tile([P, NT], f32, name="rsparts")
            for g8 in range(NG):
                wt = wtp.tile([P, 2, 4000], bf16, name="wt")
                nc.sync.dma_start(
                    out=wt[:], in_=w2_d[:, :, g8 * 4000:(g8 + 1) * 4000]
                )
                for tt in range(8):
                    t = g8 * 8 + tt
                    cols = slice(tt * TS, (tt + 1) * TS)
                    pd = pdp.tile([P, TS], f32, name="pd")
                    nc.tensor.matmul(
                        out=pd[:], lhsT=xa[:, 0, :], rhs=wt[:, 0, cols],
                        start=True, stop=False,
                    )
                    nc.tensor.matmul(
                        out=pd[:], lhsT=xa[:, 1, :], rhs=wt[:, 1, cols],
                        start=False, stop=not bias_nonzero,
                    )
                    if bias_nonzero:
                        nc.tensor.matmul(
                            out=pd[:], lhsT=onesM[t:t + 1, :],
                            rhs=brow[t:t + 1, :],
                            start=False, stop=True,
                        )
                    es = esp.tile([P, TS], bf16, name="es")
                    nc.scalar.activation(
                        out=es[:], in_=pd[:], func=AF.Exp,
                        accum_out=rsparts[:, t:t + 1],
                    )
                    nc.vector.tensor_copy(
                        out=logits_sb[:, t * TS:(t + 1) * TS], in_=pd[:]
                    )

            # ---------------- lse and per-row bias ----------------
            rs = work.tile([P, 1], f32, name="rs")
            nc.vector.reduce_sum(out=rs[:], in_=rsparts[:], axis=mybir.AxisListType.X)
            lse = work.tile([P, 1], f32, name="lse")
            nc.scalar.activation(out=lse[:], in_=rs[:], func=AF.Ln)
            rowbias = work.tile([P, 1], f32, name="rowbias")
            nc.vector.tensor_tensor(
                out=rowbias[:], in0=logg[:], in1=lse[:], op=ALU.subtract
            )

            # ---------------- sparse correction ----------------
            t1 = work.tile([P, TK], f32, name="t1")
            nc.scalar.activation(
                out=t1[:], in_=lt[:], func=AF.Exp, bias=rowbias[:, :1]
            )
            tmp = work.tile([P, TK], f32, name="tmp")
            nc.vector.tensor_scalar(
                out=tmp[:], in0=pp[:], scalar1=gm1[:, :1], scalar2=None,
                op0=ALU.mult,
            )
            nc.vector.tensor_tensor(out=tmp[:], in0=tmp[:], in1=t1[:], op=ALU.add)
            corr = work.tile([P, TK], f32, name="corr")
            nc.scalar.activation(out=corr[:], in_=tmp[:], func=AF.Ln)
            nc.sync.dma_start(out=corr_d[:], in_=corr[:])

            # ---------------- dense output writes ----------------
            for cch in range(16):
                ccols = slice(cch * 2000, (cch + 1) * 2000)
                ot = otp.tile([P, 2000], bf16, name="ot")
                nc.vector.tensor_scalar(
                    out=ot[:], in0=logits_sb[:, ccols],
                    scalar1=rowbias[:, :1], scalar2=None, op0=ALU.add,
                )
                nc.sync.dma_start(out=out_d[:, ccols], in_=ot[:])
    return nc


def _get_program(bias_nonzero):
    key = bool(bias_nonzero)
    if key not in _PROG_CACHE:
        nc = build_program(bias_nonzero=key)
        nc.compile()
        _PROG_CACHE[key] = nc
    return _PROG_CACHE[key]


def make_in_maps(inputs):
    dom = np.asarray(inputs["domainslots"], np.float32)
    xs_all = np.asarray(inputs["out_states"], np.float32)
    ctx_all = np.asarray(inputs["context"], np.float32)
    ids_all = np.asarray(inputs["context_plain"]).astype(np.int64)
    W = np.asarray(inputs["W_state"], np.float32)
    bs = np.asarray(inputs["b_state"], np.float32)
    Wg = np.asarray(inputs["W_gen"], np.float32)
    bg = np.asarray(inputs["b_gen"], np.float32)

    bias_nonzero = bool(np.any(bs != 0.0))

    w2h = np.ascontiguousarray(
        W.reshape(2, 128, V).transpose(1, 0, 2).astype(BF16)
    )
    wtah = np.zeros((V, 384), np.float32)
    wtah[:, :256] = W.T
    wtah[:, 256] = bs
    wtah = wtah.astype(BF16)
    wgh = np.ascontiguousarray(Wg.reshape(6, 128).T.astype(BF16))
    bghh = np.full((P, 1), float(np.asarray(bg).reshape(-1)[0]), np.float32)
    browh = np.ascontiguousarray(bs.reshape(NT, TS).astype(BF16))

    in_maps = []
    for i in range(8):
        xs = xs_all[2 * i:2 * i + 2].reshape(128, 256)
        xaug = np.zeros((384, 128), np.float32)
        xaug[:256] = xs.T
        xaug[256] = 1.0
        xah = np.ascontiguousarray(
            xaug.reshape(3, 128, 128).transpose(1, 0, 2).astype(BF16)
        )
        ds = dom[2 * i:2 * i + 2].reshape(128, 256)
        domh = np.ascontiguousarray(
            ds.T.reshape(2, 128, 128).transpose(1, 0, 2).astype(BF16)
        )
        cb = ctx_all[2 * i:2 * i + 2]
        ctxTh = np.ascontiguousarray(
            cb.transpose(2, 0, 1).reshape(2, 128, 2, TK)
            .transpose(1, 0, 2, 3).astype(BF16)
        )
        ctxh = np.ascontiguousarray(
            cb.reshape(2, 4, 128, D).transpose(2, 0, 1, 3).astype(BF16)
        )
        idsb = ids_all[2 * i:2 * i + 2]
        idsperm = idsb.reshape(2, 4, 128).transpose(2, 0, 1).reshape(128, 8)
        m = dict(
            xa=xah,
            domT=domh,
            ctxT=ctxTh,
            ctx=ctxh,
            w2=w2h,
            wta=wtah,
            idsf=np.ascontiguousarray(idsperm.astype(np.float32)),
            idsi=np.ascontiguousarray(idsperm.astype(np.int32)),
            wg=wgh,
            bgh=bghh,
        )
        if bias_nonzero:
            m["brow"] = browh
        in_maps.append(m)
    return in_maps, bias_nonzero


def assemble_core(dense, corr, idsb):
    """Place device-computed corrections into the dense [128, V] block.

    Duplicate ids carry bit-identical corr values, so last-write-wins is fine.
    """
    out = np.asarray(dense, dtype=np.float32)
    out[:64, idsb[0]] = corr[:64]
    out[64:, idsb[1]] = corr[64:]
    return out


def kernel(**inputs):
    global LAST_RESULTS
    in_maps, bias_nonzero = make_in_maps(inputs)
    nc = _get_program(bias_nonzero)
    trace = os.environ.get("KERNEL_TRACE", "0") == "1"
    res = bass_utils.run_bass_kernel_spmd(nc, in_maps, list(range(8)), trace=trace)
    LAST_RESULTS = res
    ids_all = np.asarray(inputs["context_plain"]).astype(np.int64)
    blocks = []
    for i in range(8):
        dense = np.asarray(res.results[i]["out"])
        corr = np.asarray(res.results[i]["corr_out"], dtype=np.float32)
        blocks.append(assemble_core(dense, corr, ids_all[2 * i:2 * i + 2]))
    return np.stack(blocks, axis=0).reshape(B, TQ, V)


# revision 19
# speedup vs baseline: 1.1336x; 1.1336x over previous
"""PointerGenerator Bass kernel for 8 TRN2 cores.

Math (per row r = (batch b, query q)):
  attn       = softmax(x_r . C_b^T / 16)                       [512]
  logits     = x_r @ W + b_state                               [32000]
  lse        = logsumexp(logits)
  g          = sigmoid([x; ctx_vec; dom] @ W_gen + b_gen)
  out dense  = logit_v + (log g - lse)            (p_ptr = 0 columns)
  out touched= log(g*exp(logit_v - lse) + (1-g)*pptr_v)  at v = ids[b,k]

Sharding: pure data parallel, 2 batches (128 rows) per core. Dense part is a
[128,32000] affine map of the logits (written bf16); the sparse correction is
computed on-device for the <=512 touched columns per row and emitted as a second
[128,512] f32 output. The host places the corrections during unshard (HW's
indirect-DMA scatter is one-offset-per-partition, so elementwise scatter is not
expressible on-device; duplicate ids carry bit-identical values so placement
order is irrelevant).
"""

import os

import numpy as np
import ml_dtypes

from concourse import bass, mybir, bass_utils
import concourse.tile as tile
from concourse.masks import make_identity

BF16 = ml_dtypes.bfloat16
F8 = ml_dtypes.float8_e4m3
B, TQ, TK, D, V = 16, 64, 512, 256, 32000
P = 128
TS = 500          # dense column tile size
NT = V // TS      # 64 dense tiles
NG = 8            # W load groups (8 tiles each)
WS = 64.0         # fp8 pre-scale on W_state
XS = 8.0          # fp8 pre-scale on out_states

LAST_RESULTS = None
_PROG_CACHE = {}


def build_program(bias_nonzero=False, debug=False):
    from concourse import bacc

    nc = bacc.Bacc(None, target_bir_lowering=False, debug=debug)
    dt = mybir.dt
    f32, bf16, i32 = dt.float32, dt.bfloat16, dt.int32
    AF = mybir.ActivationFunctionType
    ALU = mybir.AluOpType

    f8 = dt.float8e4
    xa_d = nc.dram_tensor("xa", [P, 3, P], bf16, kind="ExternalInput")
    xf8_d = nc.dram_tensor("xf8", [P, 2, P], f8, kind="ExternalInput")
    dom_d = nc.dram_tensor("domT", [P, 2, P], bf16, kind="ExternalInput")
    ctxT_d = nc.dram_tensor("ctxT", [P, 2, 2, TK], bf16, kind="ExternalInput")
    ctx_d = nc.dram_tensor("ctx", [P, 2, 4, D], bf16, kind="ExternalInput")
    w2_d = nc.dram_tensor("w2", [P, 2, V], f8, kind="ExternalInput")
    wta_d = nc.dram_tensor("wta", [V, 384], bf16, kind="ExternalInput")
    idsf_d = nc.dram_tensor("idsf", [P, 8], f32, kind="ExternalInput")
    idsi_d = nc.dram_tensor("idsi", [P, 8], i32, kind="ExternalInput")
    wg_d = nc.dram_tensor("wg", [P, 6], bf16, kind="ExternalInput")
    bgh_d = nc.dram_tensor("bgh", [P, 1], f32, kind="ExternalInput")
    if bias_nonzero:
        brow_d = nc.dram_tensor("brow", [NT, TS], bf16, kind="ExternalInput")
    out_d = nc.dram_tensor("out", [P, V], bf16, kind="ExternalOutput")
    corr_d = nc.dram_tensor("corr_out", [P, TK], f32, kind="ExternalOutput")

    with tile.TileContext(nc) as tc:
        with (
            tc.tile_pool(name="cpool", bufs=1) as cpool,
            tc.tile_pool(name="work", bufs=1) as work,
            tc.tile_pool(name="mcp", bufs=2) as mcp,
            tc.tile_pool(name="gwp", bufs=2) as gwp,
            tc.tile_pool(name="wtp", bufs=2) as wtp,
            tc.tile_pool(name="esp", bufs=2) as esp,
            tc.tile_pool(name="otp", bufs=2) as otp,
            tc.tile_pool(name="pdp", bufs=2, space="PSUM") as pdp,
            tc.tile_pool(name="ptp", bufs=2, space="PSUM") as ptp,
            tc.tile_pool(name="pmp", bufs=1, space="PSUM") as pmp,
        ):
            # ---------------- constant loads ----------------
            xa = cpool.tile([P, 3, P], bf16, name="xa_t")
            nc.sync.dma_start(out=xa[:], in_=xa_d[:])
            xf8 = cpool.tile([P, 2, P], f8, name="xf8_t")
            nc.sync.dma_start(out=xf8[:], in_=xf8_d[:])
            domT = cpool.tile([P, 2, P], bf16, name="domT_t")
            nc.sync.dma_start(out=domT[:], in_=dom_d[:])
            ctxT = cpool.tile([P, 2, 2, TK], bf16, name="ctxT_t")
            nc.sync.dma_start(out=ctxT[:], in_=ctxT_d[:])
            ctxs = cpool.tile([P, 2, 4, D], bf16, name="ctxs_t")
            nc.sync.dma_start(out=ctxs[:], in_=ctx_d[:])
            idsf = cpool.tile([P, 8], f32, name="idsf_t")
            nc.sync.dma_start(out=idsf[:], in_=idsf_d[:])
            idsi = cpool.tile([P, 8], i32, name="idsi_t")
            nc.sync.dma_start(out=idsi[:], in_=idsi_d[:])
            wg = cpool.tile([P, 6], bf16, name="wg_t")
            nc.sync.dma_start(out=wg[:], in_=wg_d[:])
            bgh = cpool.tile([P, 1], f32, name="bgh_t")
            nc.sync.dma_start(out=bgh[:], in_=bgh_d[:])
            if bias_nonzero:
                brow = cpool.tile([NT, TS], bf16, name="brow_t")
                nc.sync.dma_start(out=brow[:], in_=brow_d[:])
                onesM = cpool.tile([P, P], bf16, name="onesM")
                nc.gpsimd.memset(onesM[:], 1.0)

            ident_f = cpool.tile([P, P], f32, name="ident_f")
            make_identity(nc, ident_f[:])
            ident_b = cpool.tile([P, P], bf16, name="ident_b")
            make_identity(nc, ident_b[:])

            # ---------------- attention scores + softmax ----------------
            ps = pmp.tile([P, TK], f32, name="ps", tag="ps")
            for b in range(2):
                bc = slice(b * 64, (b + 1) * 64)
                for kc in range(2):
                    nc.tensor.matmul(
                        out=ps[bc, :],
                        lhsT=xa[:, kc, bc],
                        rhs=ctxT[:, kc, b, :],
                        start=(kc == 0),
                        stop=(kc == 1),
                    )
            attn_exp = work.tile([P, TK], f32, name="attn_exp")
            arow = work.tile([P, 1], f32, name="arow")
            nc.scalar.activation(
                out=attn_exp[:], in_=ps[:], func=AF.Exp,
                scale=1.0 / 16.0, accum_out=arow[:],
            )
            arec = work.tile([P, 1], f32, name="arec")
            nc.vector.reciprocal(arec[:], arow[:])
            attn_bf = work.tile([P, TK], bf16, name="attn_bf")
            nc.vector.tensor_scalar(
                out=attn_bf[:], in0=attn_exp[:],
                scalar1=arec[:, :1], scalar2=None, op0=ALU.mult,
            )

            # ---------------- attn transposed [key, query] ----------------
            attnT = work.tile([P, 2, 4, 64], bf16, name="attnT")
            for b in range(2):
                bc = slice(b * 64, (b + 1) * 64)
                ib = slice(b * 64, b * 64 + 64)
                for c in range(4):
                    tp = ptp.tile([P, 64], bf16, name="tp", tag="tp")
                    nc.tensor.transpose(
                        out=tp[:],
                        in_=attn_bf[bc, c * 128:(c + 1) * 128],
                        identity=ident_b[ib, ib],
                    )
                    nc.vector.tensor_copy(out=attnT[:, b, c, :], in_=tp[:])

            # ---------------- row-replicated ids ----------------
            idsrow = work.tile([P, 2, TK], f32, name="idsrow")
            for b in range(2):
                for c in range(4):
                    col = b * 4 + c
                    tp = ptp.tile([P, P], f32, name="tp", tag="tp")
                    nc.tensor.transpose(
                        out=tp[:],
                        in_=idsf[:, col:col + 1].to_broadcast([P, P]),
                        identity=ident_f[:],
                    )
                    nc.vector.tensor_copy(
                        out=idsrow[:, b, c * 128:(c + 1) * 128], in_=tp[:]
                    )

            # ------- pptr at touched cols: attn @ (ids==ids') ----------
            pp = pmp.tile([P, TK], f32, name="pp", tag="pp")
            for b in range(2):
                bc = slice(b * 64, (b + 1) * 64)
                for c in range(4):
                    col = b * 4 + c
                    mc = mcp.tile([P, TK], bf16, name="mc")
                    nc.vector.tensor_scalar(
                        out=mc[:], in0=idsrow[:, b, :],
                        scalar1=idsf[:, col:col + 1], scalar2=None,
                        op0=ALU.is_equal,
                    )
                    nc.tensor.matmul(
                        out=pp[bc, :], lhsT=attnT[:, b, c, :], rhs=mc[:],
                        start=(c == 0), stop=(c == 3),
                    )

            # ---------------- context vector + gate ----------------
            cv = pmp.tile([P, D], f32, name="cv", tag="seq")
            for b in range(2):
                bc = slice(b * 64, (b + 1) * 64)
                for c in range(4):
                    nc.tensor.matmul(
                        out=cv[bc, :], lhsT=attnT[:, b, c, :],
                        rhs=ctxs[:, b, c, :],
                        start=(c == 0), stop=(c == 3),
                    )
            cvec_sb = work.tile([P, D], bf16, name="cvec_sb")
            nc.vector.tensor_copy(out=cvec_sb[:], in_=cv[:])

            cvecT = work.tile([P, 2, 2, 64], bf16, name="cvecT")
            for b in range(2):
                bc = slice(b * 64, (b + 1) * 64)
                ib = slice(b * 64, b * 64 + 64)
                for kc in range(2):
                    tp = ptp.tile([P, 64], bf16, name="tp", tag="tp")
                    nc.tensor.transpose(
                        out=tp[:],
                        in_=cvec_sb[bc, kc * 128:(kc + 1) * 128],
                        identity=ident_b[ib, ib],
                    )
                    nc.vector.tensor_copy(out=cvecT[:, b, kc, :], in_=tp[:])

            gp = pmp.tile([P, 1], f32, name="gp", tag="seq")
            for b in range(2):
                bc = slice(b * 64, (b + 1) * 64)
                lhs = [
                    xa[:, 0, bc], xa[:, 1, bc],
                    cvecT[:, b, 0, :], cvecT[:, b, 1, :],
                    domT[:, 0, bc], domT[:, 1, bc],
                ]
                for j, lh in enumerate(lhs):
                    nc.tensor.matmul(
                        out=gp[bc, :], lhsT=lh, rhs=wg[:, j:j + 1],
                        start=(j == 0), stop=(j == 5),
                    )
            g = work.tile([P, 1], f32, name="g")
            nc.scalar.activation(out=g[:], in_=gp[:], func=AF.Sigmoid, bias=bgh[:, :1])
            logg = work.tile([P, 1], f32, name="logg")
            nc.scalar.activation(out=logg[:], in_=g[:], func=AF.Ln)
            gm1 = work.tile([P, 1], f32, name="gm1")
            nc.vector.tensor_scalar(
                out=gm1[:], in0=g[:], scalar1=-1.0, scalar2=1.0,
                op0=ALU.mult, op1=ALU.add,
            )

            # -------- gather W columns at touched ids; logits there --------
            wcolsT = work.tile([P, 2, 3, TK], bf16, name="wcolsT")
            for b in range(2):
                for c in range(4):
                    col = b * 4 + c
                    gw = gwp.tile([P, 384], bf16, name="gw")
                    nc.gpsimd.indirect_dma_start(
                        out=gw[:],
                        out_offset=None,
                        in_=wta_d[:],
                        in_offset=bass.IndirectOffsetOnAxis(
                            ap=idsi[:, col:col + 1], axis=0
                        ),
                    )
                    for f in range(3):
                        tp = ptp.tile([P, P], bf16, name="tp", tag="tp")
                        nc.tensor.transpose(
                            out=tp[:],
                            in_=gw[:, f * 128:(f + 1) * 128],
                            identity=ident_b[:],
                        )
                        nc.vector.tensor_copy(
                            out=wcolsT[:, b, f, c * 128:(c + 1) * 128], in_=tp[:]
                        )
            lt = pmp.tile([P, TK], f32, name="lt", tag="seq")
            for b in range(2):
                bc = slice(b * 64, (b + 1) * 64)
                for f in range(3):
                    nc.tensor.matmul(
                        out=lt[bc, :], lhsT=xa[:, f, bc], rhs=wcolsT[:, b, f, :],
                        start=(f == 0), stop=(f == 2),
                    )

            # ---------------- dense logits + exp-sums ----------------
            logits_sb = work.tile([P, V], bf16, name="logits_sb")
            rsparts = work.tile([P, NT], f32, name="rsparts")
            sinv = 1.0 / (WS * XS)
            for g8 in range(NG):
                wt = wtp.tile([P, 2, 4000], f8, name="wt")
                nc.sync.dma_start(
                    out=wt[:], in_=w2_d[:, :, g8 * 4000:(g8 + 1) * 4000]
                )
                for tt in range(8):
                    t = g8 * 8 + tt
                    cols = slice(tt * TS, (tt + 1) * TS)
                    pd = pdp.tile([P, TS], f32, name="pd")
                    nc.tensor.matmul(
                        out=pd[:], lhsT=xf8[:, :, :], rhs=wt[:, :, cols],
                        start=True, stop=not bias_nonzero,
                        perf_mode=mybir.MatmulPerfMode.DoubleRow,
                    )
                    if bias_nonzero:
                        nc.tensor.matmul(
                            out=pd[:], lhsT=onesM[t:t + 1, :],
                            rhs=brow[t:t + 1, :],
                            start=False, stop=True,
                        )
                    es = esp.tile([P, TS], bf16, name="es")
                    nc.scalar.activation(
                        out=es[:], in_=pd[:], func=AF.Exp, scale=sinv,
                        accum_out=rsparts[:, t:t + 1],
                    )
                    nc.vector.tensor_scalar(
                        out=logits_sb[:, t * TS:(t + 1) * TS], in0=pd[:],
                        scalar1=sinv, scalar2=None, op0=ALU.mult,
                    )

            # ---------------- lse and per-row bias ----------------
            rs = work.tile([P, 1], f32, name="rs")
            nc.vector.reduce_sum(out=rs[:], in_=rsparts[:], axis=mybir.AxisListType.X)
            lse = work.tile([P, 1], f32, name="lse")
            nc.scalar.activation(out=lse[:], in_=rs[:], func=AF.Ln)
            rowbias = work.tile([P, 1], f32, name="rowbias")
            nc.vector.tensor_tensor(
                out=rowbias[:], in0=logg[:], in1=lse[:], op=ALU.subtract
            )

            # ---------------- sparse correction ----------------
            t1 = work.tile([P, TK], f32, name="t1")
            nc.scalar.activation(
                out=t1[:], in_=lt[:], func=AF.Exp, bias=rowbias[:, :1]
            )
            tmp = work.tile([P, TK], f32, name="tmp")
            nc.vector.tensor_scalar(
                out=tmp[:], in0=pp[:], scalar1=gm1[:, :1], scalar2=None,
                op0=ALU.mult,
            )
            nc.vector.tensor_tensor(out=tmp[:], in0=tmp[:], in1=t1[:], op=ALU.add)
            corr = work.tile([P, TK], f32, name="corr")
            nc.scalar.activation(out=corr[:], in_=tmp[:], func=AF.Ln)
            nc.sync.dma_start(out=corr_d[:], in_=corr[:])

            # ---------------- dense output writes ----------------
            for cch in range(16):
                ccols = slice(cch * 2000, (cch + 1) * 2000)
                ot = otp.tile([P, 2000], bf16, name="ot")
                nc.vector.tensor_scalar(
                    out=ot[:], in0=logits_sb[:, ccols],
                    scalar1=rowbias[:, :1], scalar2=None, op0=ALU.add,
                )
                nc.sync.dma_start(out=out_d[:, ccols], in_=ot[:])
    return nc


def _get_program(bias_nonzero):
    key = bool(bias_nonzero)
    if key not in _PROG_CACHE:
        nc = build_program(bias_nonzero=key)
        nc.compile()
        _PROG_CACHE[key] = nc
    return _PROG_CACHE[key]


def make_in_maps(inputs):
    dom = np.asarray(inputs["domainslots"], np.float32)
    xs_all = np.asarray(inputs["out_states"], np.float32)
    ctx_all = np.asarray(inputs["context"], np.float32)
    ids_all = np.asarray(inputs["context_plain"]).astype(np.int64)
    W = np.asarray(inputs["W_state"], np.float32)
    bs = np.asarray(inputs["b_state"], np.float32)
    Wg = np.asarray(inputs["W_gen"], np.float32)
    bg = np.asarray(inputs["b_gen"], np.float32)

    bias_nonzero = bool(np.any(bs != 0.0))

    w2h = np.ascontiguousarray(
        (W * WS).reshape(2, 128, V).transpose(1, 0, 2).astype(F8)
    )
    wtah = np.zeros((V, 384), np.float32)
    wtah[:, :256] = W.T
    wtah[:, 256] = bs
    wtah = wtah.astype(BF16)
    wgh = np.ascontiguousarray(Wg.reshape(6, 128).T.astype(BF16))
    bghh = np.full((P, 1), float(np.asarray(bg).reshape(-1)[0]), np.float32)
    browh = np.ascontiguousarray(bs.reshape(NT, TS).astype(BF16))

    in_maps = []
    for i in range(8):
        xs = xs_all[2 * i:2 * i + 2].reshape(128, 256)
        xaug = np.zeros((384, 128), np.float32)
        xaug[:256] = xs.T
        xaug[256] = 1.0
        xah = np.ascontiguousarray(
            xaug.reshape(3, 128, 128).transpose(1, 0, 2).astype(BF16)
        )
        xf8h = np.ascontiguousarray(
            (xaug[:256] * XS).reshape(2, 128, 128).transpose(1, 0, 2).astype(F8)
        )
        ds = dom[2 * i:2 * i + 2].reshape(128, 256)
        domh = np.ascontiguousarray(
            ds.T.reshape(2, 128, 128).transpose(1, 0, 2).astype(BF16)
        )
        cb = ctx_all[2 * i:2 * i + 2]
        ctxTh = np.ascontiguousarray(
            cb.transpose(2, 0, 1).reshape(2, 128, 2, TK)
            .transpose(1, 0, 2, 3).astype(BF16)
        )
        ctxh = np.ascontiguousarray(
            cb.reshape(2, 4, 128, D).transpose(2, 0, 1, 3).astype(BF16)
        )
        idsb = ids_all[2 * i:2 * i + 2]
        idsperm = idsb.reshape(2, 4, 128).transpose(2, 0, 1).reshape(128, 8)
        m = dict(
            xa=xah,
            xf8=xf8h,
            domT=domh,
            ctxT=ctxTh,
            ctx=ctxh,
            w2=w2h,
            wta=wtah,
            idsf=np.ascontiguousarray(idsperm.astype(np.float32)),
            idsi=np.ascontiguousarray(idsperm.astype(np.int32)),
            wg=wgh,
            bgh=bghh,
        )
        if bias_nonzero:
            m["brow"] = browh
        in_maps.append(m)
    return in_maps, bias_nonzero


def assemble_core(dense, corr, idsb):
    """Place device-computed corrections into the dense [128, V] block.

    Duplicate ids carry bit-identical corr values, so last-write-wins is fine.
    """
    out = np.asarray(dense, dtype=np.float32)
    out[:64, idsb[0]] = corr[:64]
    out[64:, idsb[1]] = corr[64:]
    return out


def kernel(**inputs):
    global LAST_RESULTS
    in_maps, bias_nonzero = make_in_maps(inputs)
    nc = _get_program(bias_nonzero)
    trace = os.environ.get("KERNEL_TRACE", "0") == "1"
    res = bass_utils.run_bass_kernel_spmd(nc, in_maps, list(range(8)), trace=trace)
    LAST_RESULTS = res
    ids_all = np.asarray(inputs["context_plain"]).astype(np.int64)
    blocks = []
    for i in range(8):
        dense = np.asarray(res.results[i]["out"])
        corr = np.asarray(res.results[i]["corr_out"], dtype=np.float32)
        blocks.append(assemble_core(dense, corr, ids_all[2 * i:2 * i + 2]))
    return np.stack(blocks, axis=0).reshape(B, TQ, V)


# revision 20
# speedup vs baseline: 1.3263x; 1.1700x over previous
"""PointerGenerator Bass kernel for 8 TRN2 cores.

Math (per row r = (batch b, query q)):
  attn       = softmax(x_r . C_b^T / 16)                       [512]
  logits     = x_r @ W + b_state                               [32000]
  lse        = logsumexp(logits)
  g          = sigmoid([x; ctx_vec; dom] @ W_gen + b_gen)
  out dense  = logit_v + (log g - lse)            (p_ptr = 0 columns)
  out touched= log(g*exp(logit_v - lse) + (1-g)*pptr_v)  at v = ids[b,k]

Sharding: pure data parallel, 2 batches (128 rows) per core. Dense part is a
[128,32000] affine map of the logits (written bf16); the sparse correction is
computed on-device for the <=512 touched columns per row and emitted as a second
[128,512] f32 output. The host places the corrections during unshard (HW's
indirect-DMA scatter is one-offset-per-partition, so elementwise scatter is not
expressible on-device; duplicate ids carry bit-identical values so placement
order is irrelevant).

v4: dense loop processes 1000-col chunk pairs in [P,2,512] psum tiles (one wide
ACT Exp + one wide DVE copy per pair), W groups prefetched 3-deep, scalar ops
ordered for 4 ACT table loads, rowbias = Ln(g/rs) (one Ln replaces logg/lse).
"""

import os

import numpy as np
import ml_dtypes

from concourse import bass, mybir, bass_utils
import concourse.tile as tile
from concourse.masks import make_identity

BF16 = ml_dtypes.bfloat16
F8 = ml_dtypes.float8_e4m3
B, TQ, TK, D, V = 16, 64, 512, 256, 32000
P = 128
TS = 500          # dense column sub-tile size
NT = V // TS      # 64 dense sub-tiles (bias rows)
NT2 = 32          # 1000-col chunk pairs
NG = 8            # W load groups (4 chunk pairs each)
WS = 64.0         # fp8 pre-scale on W_state
XS = 8.0          # fp8 pre-scale on out_states

LAST_RESULTS = None
_PROG_CACHE = {}


def build_program(bias_nonzero=False, debug=False):
    from concourse import bacc

    nc = bacc.Bacc(None, target_bir_lowering=False, debug=debug)
    dt = mybir.dt
    f32, bf16, i32 = dt.float32, dt.bfloat16, dt.int32
    AF = mybir.ActivationFunctionType
    ALU = mybir.AluOpType

    f8 = dt.float8e4
    xa_d = nc.dram_tensor("xa", [P, 3, P], bf16, kind="ExternalInput")
    xf8_d = nc.dram_tensor("xf8", [P, 2, P], f8, kind="ExternalInput")
    dom_d = nc.dram_tensor("domT", [P, 2, P], bf16, kind="ExternalInput")
    ctxT_d = nc.dram_tensor("ctxT", [P, 2, 2, TK], bf16, kind="ExternalInput")
    ctx_d = nc.dram_tensor("ctx", [P, 2, 4, D], bf16, kind="ExternalInput")
    w2_d = nc.dram_tensor("w2", [P, 2, V], f8, kind="ExternalInput")
    wta_d = nc.dram_tensor("wta", [V, 384], bf16, kind="ExternalInput")
    idsf_d = nc.dram_tensor("idsf", [P, 8], f32, kind="ExternalInput")
    idsi_d = nc.dram_tensor("idsi", [P, 8], i32, kind="ExternalInput")
    wg_d = nc.dram_tensor("wg", [P, 6], bf16, kind="ExternalInput")
    bgh_d = nc.dram_tensor("bgh", [P, 1], f32, kind="ExternalInput")
    if bias_nonzero:
        brow_d = nc.dram_tensor("brow", [NT, TS], bf16, kind="ExternalInput")
    out_d = nc.dram_tensor("out", [P, NT2, 2, TS], bf16, kind="ExternalOutput")
    corr_d = nc.dram_tensor("corr_out", [P, TK], f32, kind="ExternalOutput")

    with tile.TileContext(nc) as tc:
        with (
            tc.tile_pool(name="cpool", bufs=1) as cpool,
            tc.tile_pool(name="work", bufs=1) as work,
            tc.tile_pool(name="mcp", bufs=2) as mcp,
            tc.tile_pool(name="gwp", bufs=2) as gwp,
            tc.tile_pool(name="wtp", bufs=3) as wtp,
            tc.tile_pool(name="esp", bufs=2) as esp,
            tc.tile_pool(name="otp", bufs=2) as otp,
            tc.tile_pool(name="pdp", bufs=2, space="PSUM") as pdp,
            tc.tile_pool(name="ptp", bufs=2, space="PSUM") as ptp,
            tc.tile_pool(name="pmp", bufs=1, space="PSUM") as pmp,
        ):
            # ---------------- constant loads ----------------
            xa = cpool.tile([P, 3, P], bf16, name="xa_t")
            nc.sync.dma_start(out=xa[:], in_=xa_d[:])
            xf8 = cpool.tile([P, 2, P], f8, name="xf8_t")
            nc.sync.dma_start(out=xf8[:], in_=xf8_d[:])
            domT = cpool.tile([P, 2, P], bf16, name="domT_t")
            nc.sync.dma_start(out=domT[:], in_=dom_d[:])
            ctxT = cpool.tile([P, 2, 2, TK], bf16, name="ctxT_t")
            nc.sync.dma_start(out=ctxT[:], in_=ctxT_d[:])
            ctxs = cpool.tile([P, 2, 4, D], bf16, name="ctxs_t")
            nc.sync.dma_start(out=ctxs[:], in_=ctx_d[:])
            idsf = cpool.tile([P, 8], f32, name="idsf_t")
            nc.sync.dma_start(out=idsf[:], in_=idsf_d[:])
            idsi = cpool.tile([P, 8], i32, name="idsi_t")
            nc.sync.dma_start(out=idsi[:], in_=idsi_d[:])
            wg = cpool.tile([P, 6], bf16, name="wg_t")
            nc.sync.dma_start(out=wg[:], in_=wg_d[:])
            bgh = cpool.tile([P, 1], f32, name="bgh_t")
            nc.sync.dma_start(out=bgh[:], in_=bgh_d[:])
            if bias_nonzero:
                brow = cpool.tile([NT, TS], bf16, name="brow_t")
                nc.sync.dma_start(out=brow[:], in_=brow_d[:])
                onesM = cpool.tile([P, P], bf16, name="onesM")
                nc.gpsimd.memset(onesM[:], 1.0)

            # ---------------- W group prefetch (3 deep) ----------------
            wts = {}

            def load_group(g8):
                wt = wtp.tile([P, 2, 4000], f8, name="wt")
                nc.sync.dma_start(
                    out=wt[:], in_=w2_d[:, :, g8 * 4000:(g8 + 1) * 4000]
                )
                wts[g8] = wt

            load_group(0)
            load_group(1)
            load_group(2)

            ident_f = cpool.tile([P, P], f32, name="ident_f")
            make_identity(nc, ident_f[:])
            ident_b = cpool.tile([P, P], bf16, name="ident_b")
            make_identity(nc, ident_b[:])

            # ---------------- attention scores + softmax ----------------
            ps = pmp.tile([P, TK], f32, name="ps", tag="a")
            for b in range(2):
                bc = slice(b * 64, (b + 1) * 64)
                for kc in range(2):
                    nc.tensor.matmul(
                        out=ps[bc, :],
                        lhsT=xa[:, kc, bc],
                        rhs=ctxT[:, kc, b, :],
                        start=(kc == 0),
                        stop=(kc == 1),
                    )
            attn_exp = work.tile([P, TK], f32, name="attn_exp")
            arow = work.tile([P, 1], f32, name="arow")
            nc.scalar.activation(
                out=attn_exp[:], in_=ps[:], func=AF.Exp,
                scale=1.0 / 16.0, accum_out=arow[:],
            )
            arec = work.tile([P, 1], f32, name="arec")
            nc.vector.reciprocal(arec[:], arow[:])
            attn_bf = work.tile([P, TK], bf16, name="attn_bf")
            nc.vector.tensor_scalar(
                out=attn_bf[:], in0=attn_exp[:],
                scalar1=arec[:, :1], scalar2=None, op0=ALU.mult,
            )

            # ---------------- attn transposed [key, query] ----------------
            attnT = work.tile([P, 2, 4, 64], bf16, name="attnT")
            for b in range(2):
                bc = slice(b * 64, (b + 1) * 64)
                ib = slice(b * 64, b * 64 + 64)
                for c in range(4):
                    tp = ptp.tile([P, 64], bf16, name="tp", tag="tp")
                    nc.tensor.transpose(
                        out=tp[:],
                        in_=attn_bf[bc, c * 128:(c + 1) * 128],
                        identity=ident_b[ib, ib],
                    )
                    nc.vector.tensor_copy(out=attnT[:, b, c, :], in_=tp[:])

            # ---------------- context vector ----------------
            cv = pmp.tile([P, D], f32, name="cv", tag="a")
            for b in range(2):
                bc = slice(b * 64, (b + 1) * 64)
                for c in range(4):
                    nc.tensor.matmul(
                        out=cv[bc, :], lhsT=attnT[:, b, c, :],
                        rhs=ctxs[:, b, c, :],
                        start=(c == 0), stop=(c == 3),
                    )
            cvec_sb = work.tile([P, D], bf16, name="cvec_sb")
            nc.vector.tensor_copy(out=cvec_sb[:], in_=cv[:])

            cvecT = work.tile([P, 2, 2, 64], bf16, name="cvecT")
            for b in range(2):
                bc = slice(b * 64, (b + 1) * 64)
                ib = slice(b * 64, b * 64 + 64)
                for kc in range(2):
                    tp = ptp.tile([P, 64], bf16, name="tp", tag="tp")
                    nc.tensor.transpose(
                        out=tp[:],
                        in_=cvec_sb[bc, kc * 128:(kc + 1) * 128],
                        identity=ident_b[ib, ib],
                    )
                    nc.vector.tensor_copy(out=cvecT[:, b, kc, :], in_=tp[:])

            # ---------------- row-replicated ids ----------------
            idsrow = work.tile([P, 2, TK], f32, name="idsrow")
            for b in range(2):
                for c in range(4):
                    col = b * 4 + c
                    tp = ptp.tile([P, P], f32, name="tp", tag="tp")
                    nc.tensor.transpose(
                        out=tp[:],
                        in_=idsf[:, col:col + 1].to_broadcast([P, P]),
                        identity=ident_f[:],
                    )
                    nc.vector.tensor_copy(
                        out=idsrow[:, b, c * 128:(c + 1) * 128], in_=tp[:]
                    )

            # ------- pptr at touched cols: attn @ (ids==ids') ----------
            pp = pmp.tile([P, TK], f32, name="pp", tag="a")
            for b in range(2):
                bc = slice(b * 64, (b + 1) * 64)
                for c in range(4):
                    col = b * 4 + c
                    mc = mcp.tile([P, TK], bf16, name="mc")
                    nc.vector.tensor_scalar(
                        out=mc[:], in0=idsrow[:, b, :],
                        scalar1=idsf[:, col:col + 1], scalar2=None,
                        op0=ALU.is_equal,
                    )
                    nc.tensor.matmul(
                        out=pp[bc, :], lhsT=attnT[:, b, c, :], rhs=mc[:],
                        start=(c == 0), stop=(c == 3),
                    )

            # ---------------- gate ----------------
            gp = pmp.tile([P, 1], f32, name="gp", tag="c")
            for b in range(2):
                bc = slice(b * 64, (b + 1) * 64)
                lhs = [
                    xa[:, 0, bc], xa[:, 1, bc],
                    cvecT[:, b, 0, :], cvecT[:, b, 1, :],
                    domT[:, 0, bc], domT[:, 1, bc],
                ]
                for j, lh in enumerate(lhs):
                    nc.tensor.matmul(
                        out=gp[bc, :], lhsT=lh, rhs=wg[:, j:j + 1],
                        start=(j == 0), stop=(j == 5),
                    )
            g = work.tile([P, 1], f32, name="g")
            nc.scalar.activation(out=g[:], in_=gp[:], func=AF.Sigmoid, bias=bgh[:, :1])
            gm1 = work.tile([P, 1], f32, name="gm1")
            nc.vector.tensor_scalar(
                out=gm1[:], in0=g[:], scalar1=-1.0, scalar2=1.0,
                op0=ALU.mult, op1=ALU.add,
            )

            # -------- gather W columns at touched ids; logits there --------
            wcolsT = work.tile([P, 2, 3, TK], bf16, name="wcolsT")
            for b in range(2):
                for c in range(4):
                    col = b * 4 + c
                    gw = gwp.tile([P, 384], bf16, name="gw")
                    nc.gpsimd.indirect_dma_start(
                        out=gw[:],
                        out_offset=None,
                        in_=wta_d[:],
                        in_offset=bass.IndirectOffsetOnAxis(
                            ap=idsi[:, col:col + 1], axis=0
                        ),
                    )
                    for f in range(3):
                        tp = ptp.tile([P, P], bf16, name="tp", tag="tp")
                        nc.tensor.transpose(
                            out=tp[:],
                            in_=gw[:, f * 128:(f + 1) * 128],
                            identity=ident_b[:],
                        )
                        nc.vector.tensor_copy(
                            out=wcolsT[:, b, f, c * 128:(c + 1) * 128], in_=tp[:]
                        )
            lt = pmp.tile([P, TK], f32, name="lt", tag="c")
            for b in range(2):
                bc = slice(b * 64, (b + 1) * 64)
                for f in range(3):
                    nc.tensor.matmul(
                        out=lt[bc, :], lhsT=xa[:, f, bc], rhs=wcolsT[:, b, f, :],
                        start=(f == 0), stop=(f == 2),
                    )
            # exp(logit at touched cols), before lse is known
            t1 = work.tile([P, TK], f32, name="t1")
            nc.scalar.activation(out=t1[:], in_=lt[:], func=AF.Exp)

            # ---------------- dense logits + exp-sums ----------------
            logits_sb = work.tile([P, NT2, 2, TS], bf16, name="logits_sb")
            rsparts = work.tile([P, NT2], f32, name="rsparts")
            sinv = 1.0 / (WS * XS)
            for g8 in range(NG):
                wt = wts[g8]
                for cc in range(4):
                    t = g8 * 4 + cc
                    pd = pdp.tile([P, 2, 512], f32, name="pd")
                    for j in range(2):
                        cols = slice((cc * 2 + j) * TS, (cc * 2 + j + 1) * TS)
                        nc.tensor.matmul(
                            out=pd[:, j, 0:TS], lhsT=xf8[:, :, :],
                            rhs=wt[:, :, cols],
                            start=True, stop=not bias_nonzero,
                            perf_mode=mybir.MatmulPerfMode.DoubleRow,
                        )
                        if bias_nonzero:
                            tt = g8 * 8 + cc * 2 + j
                            nc.tensor.matmul(
                                out=pd[:, j, 0:TS], lhsT=onesM[tt:tt + 1, :],
                                rhs=brow[tt:tt + 1, :],
                                start=False, stop=True,
                            )
                    es = esp.tile([P, 2, TS], bf16, name="es")
                    nc.scalar.activation(
                        out=es[:], in_=pd[:, :, 0:TS], func=AF.Exp, scale=sinv,
                        accum_out=rsparts[:, t:t + 1],
                    )
                    nc.vector.tensor_scalar(
                        out=logits_sb[:, t, :, :], in0=pd[:, :, 0:TS],
                        scalar1=sinv, scalar2=None, op0=ALU.mult,
                    )
                if g8 + 3 < NG:
                    load_group(g8 + 3)

            # ---------------- lse and per-row bias ----------------
            rs = work.tile([P, 1], f32, name="rs")
            nc.vector.reduce_sum(out=rs[:], in_=rsparts[:], axis=mybir.AxisListType.X)
            rrec = work.tile([P, 1], f32, name="rrec")
            nc.vector.reciprocal(rrec[:], rs[:])
            grr = work.tile([P, 1], f32, name="grr")
            nc.vector.tensor_tensor(out=grr[:], in0=g[:], in1=rrec[:], op=ALU.mult)
            rowbias = work.tile([P, 1], f32, name="rowbias")
            nc.scalar.activation(out=rowbias[:], in_=grr[:], func=AF.Ln)

            # ---------------- sparse correction ----------------
            # corr = ln(gm1*pp + t1 * g/rs)
            t1g = work.tile([P, TK], f32, name="t1g")
            nc.vector.tensor_scalar(
                out=t1g[:], in0=t1[:], scalar1=grr[:, :1], scalar2=None,
                op0=ALU.mult,
            )
            tmp = work.tile([P, TK], f32, name="tmp")
            nc.vector.tensor_scalar(
                out=tmp[:], in0=pp[:], scalar1=gm1[:, :1], scalar2=None,
                op0=ALU.mult,
            )
            nc.vector.tensor_tensor(out=tmp[:], in0=tmp[:], in1=t1g[:], op=ALU.add)
            corr = work.tile([P, TK], f32, name="corr")
            nc.scalar.activation(out=corr[:], in_=tmp[:], func=AF.Ln)
            nc.sync.dma_start(out=corr_d[:], in_=corr[:])

            # ---------------- dense output writes ----------------
            for cch in range(16):
                csl = slice(cch * 2, (cch + 1) * 2)
                ot = otp.tile([P, 2, 2, TS], bf16, name="ot")
                nc.vector.tensor_scalar(
                    out=ot[:], in0=logits_sb[:, csl, :, :],
                    scalar1=rowbias[:, :1], scalar2=None, op0=ALU.add,
                )
                nc.sync.dma_start(out=out_d[:, csl, :, :], in_=ot[:])
    return nc


def _get_program(bias_nonzero):
    key = bool(bias_nonzero)
    if key not in _PROG_CACHE:
        nc = build_program(bias_nonzero=key)
        nc.compile()
        _PROG_CACHE[key] = nc
    return _PROG_CACHE[key]


def make_in_maps(inputs):
    dom = np.asarray(inputs["domainslots"], np.float32)
    xs_all = np.asarray(inputs["out_states"], np.float32)
    ctx_all = np.asarray(inputs["context"], np.float32)
    ids_all = np.asarray(inputs["context_plain"]).astype(np.int64)
    W = np.asarray(inputs["W_state"], np.float32)
    bs = np.asarray(inputs["b_state"], np.float32)
    Wg = np.asarray(inputs["W_gen"], np.float32)
    bg = np.asarray(inputs["b_gen"], np.float32)

    bias_nonzero = bool(np.any(bs != 0.0))

    w2h = np.ascontiguousarray(
        (W * WS).reshape(2, 128, V).transpose(1, 0, 2).astype(F8)
    )
    wtah = np.zeros((V, 384), np.float32)
    wtah[:, :256] = W.T
    wtah[:, 256] = bs
    wtah = wtah.astype(BF16)
    wgh = np.ascontiguousarray(Wg.reshape(6, 128).T.astype(BF16))
    bghh = np.full((P, 1), float(np.asarray(bg).reshape(-1)[0]), np.float32)
    browh = np.ascontiguousarray(bs.reshape(NT, TS).astype(BF16))

    in_maps = []
    for i in range(8):
        xs = xs_all[2 * i:2 * i + 2].reshape(128, 256)
        xaug = np.zeros((384, 128), np.float32)
        xaug[:256] = xs.T
        xaug[256] = 1.0
        xah = np.ascontiguousarray(
            xaug.reshape(3, 128, 128).transpose(1, 0, 2).astype(BF16)
        )
        xf8h = np.ascontiguousarray(
            (xaug[:256] * XS).reshape(2, 128, 128).transpose(1, 0, 2).astype(F8)
        )
        ds = dom[2 * i:2 * i + 2].reshape(128, 256)
        domh = np.ascontiguousarray(
            ds.T.reshape(2, 128, 128).transpose(1, 0, 2).astype(BF16)
        )
        cb = ctx_all[2 * i:2 * i + 2]
        ctxTh = np.ascontiguousarray(
            cb.transpose(2, 0, 1).reshape(2, 128, 2, TK)
            .transpose(1, 0, 2, 3).astype(BF16)
        )
        ctxh = np.ascontiguousarray(
            cb.reshape(2, 4, 128, D).transpose(2, 0, 1, 3).astype(BF16)
        )
        idsb = ids_all[2 * i:2 * i + 2]
        idsperm = idsb.reshape(2, 4, 128).transpose(2, 0, 1).reshape(128, 8)
        m = dict(
            xa=xah,
            xf8=xf8h,
            domT=domh,
            ctxT=ctxTh,
            ctx=ctxh,
            w2=w2h,
            wta=wtah,
            idsf=np.ascontiguousarray(idsperm.astype(np.float32)),
            idsi=np.ascontiguousarray(idsperm.astype(np.int32)),
            wg=wgh,
            bgh=bghh,
        )
        if bias_nonzero:
            m["brow"] = browh
        in_maps.append(m)
    return in_maps, bias_nonzero


def assemble_core(dense, corr, idsb):
    """Place device-computed corrections into the dense [128, V] block.

    Duplicate ids carry bit-identical corr values, so last-write-wins is fine.
    """
    out = np.asarray(dense, dtype=np.float32).reshape(P, V)
    out[:64, idsb[0]] = corr[:64]
    out[64:, idsb[1]] = corr[64:]
    return out


def kernel(**inputs):
    global LAST_RESULTS
    in_maps, bias_nonzero = make_in_maps(inputs)
    nc = _get_program(bias_nonzero)
    trace = os.environ.get("KERNEL_TRACE", "0") == "1"
    res = bass_utils.run_bass_kernel_spmd(nc, in_maps, list(range(8)), trace=trace)
    LAST_RESULTS = res
    ids_all = np.asarray(inputs["context_plain"]).astype(np.int64)
    blocks = []
    for i in range(8):
        dense = np.asarray(res.results[i]["out"])
        corr = np.asarray(res.results[i]["corr_out"], dtype=np.float32)
        blocks.append(assemble_core(dense, corr, ids_all[2 * i:2 * i + 2]))
    return np.stack(blocks, axis=0).reshape(B, TQ, V)
